# revision 6
# baseline (speedup 1.0000x reference)
"""Trainium2 Bass kernel for causal multi-head attention with RoPE.

Model: B=2, T=2048, C=2048, H=16 heads, D=128 head_dim.
  qkv = x @ w_qkv ; q,k rotary-embedded ; causal softmax attention ; out @ w_out.

Sharding: tensor-parallel over heads. 16 heads / 8 cores = 2 heads per core.
Each core gets w_qkv columns and w_out rows for its 2 heads, computes a full
(B*T, C) partial output projection, and the host sums the 8 partials.

Per-core dataflow (matmul operands in float32r = full-rate rounded fp32):
  - x is fed pre-transposed (xT, [C, B*T]) so the C contraction sits on
    partitions.  qT/kT come out of the projection directly in [D, T] layout
    (D on partitions), v in natural [T, D] layout.
  - RoPE on DVE in [D, T] layout: rot_half is a partition-half swap done with
    two ACT copies, the sign folded into the sin table host-side.
  - scores computed transposed (sT[tk, tq] = kT.T @ qT), exp on ACT with the
    1/sqrt(D) scale folded in; causal mask additive on diagonal squares only,
    fully-masked column ranges zeroed via Copy(scale=0).
  - out_un[d, tq] accumulates v.T @ expT on PE; colsum via ones-column matmul.
  - softmax normalization deferred to the out_un PSUM->SBUF copy, multiplying
    by a partition-broadcast reciprocal colsum row (gpsimd broadcast).
  - output projection contracts the 2 local heads, interleaved per tq block,
    streamed straight to HBM.
"""

import numpy as np

import concourse.bass as bass
import concourse.tile as tile
import concourse.mybir as mybir
from concourse import bacc
from concourse.bass import ds
from concourse.bass_utils import run_bass_kernel_spmd

B, T, C, H, D = 2, 2048, 2048, 16, 128
NCORES = 8
HPC = H // NCORES  # heads per core = 2
S = B * T  # 4096 tokens
NBLK = T // 512  # 4 tq/tok blocks per batch
NCT = C // 128  # 16 contraction tiles for the qkv projection
NTK = T // 128  # 16 tk tiles per batch
F32 = mybir.dt.float32
F32R = mybir.dt.float32r
EXP_SCALE = float(D) ** -0.5
NEG = -1.0e30

_CACHE = {}


def build_nc():
    nc = bacc.Bacc("TRN2", target_bir_lowering=False, debug=False, num_devices=NCORES)

    xt_d = nc.dram_tensor("xt", [C, S], F32R, kind="ExternalInput").ap()
    wqkv_d = nc.dram_tensor("wqkv", [C, 6 * D], F32R, kind="ExternalInput").ap()
    wout_d = nc.dram_tensor("wout", [HPC * D, C], F32R, kind="ExternalInput").ap()
    cos_d = nc.dram_tensor("cos2t", [D, T], F32, kind="ExternalInput").ap()
    sin_d = nc.dram_tensor("sin2t", [D, T], F32, kind="ExternalInput").ap()
    mask_d = nc.dram_tensor("maskadd", [128, 128], F32, kind="ExternalInput").ap()
    ones_d = nc.dram_tensor("ones_in", [128, 1], F32R, kind="ExternalInput").ap()
    y_d = nc.dram_tensor("y", [S, C], F32, kind="ExternalOutput").ap()

    xt_t = xt_d.rearrange("(ct p) s -> p ct s", p=128)  # [128, 16, 4096]
    wqkv_t = wqkv_d.rearrange("(ct p) n -> p ct n", p=128)  # [128, 16, 768]
    wout_t = wout_d.rearrange("(h p) n -> p h n", p=128)  # [128, 2, 2048]

    Exp = mybir.ActivationFunctionType.Exp
    Copy = mybir.ActivationFunctionType.Copy

    with tile.TileContext(nc) as tc:
        with (
            tc.tile_pool(name="s1", bufs=1) as s1,
            tc.tile_pool(name="s2", bufs=2) as s2,
            tc.tile_pool(name="se", bufs=3) as se,
            tc.tile_pool(name="sy", bufs=2) as sy,
            tc.tile_pool(name="sou", bufs=4) as sou,
            tc.tile_pool(name="ps2", bufs=2, space="PSUM") as ps2,
            tc.tile_pool(name="ps1", bufs=1, space="PSUM") as ps1,
        ):
            # ---- resident constants ----
            wqkv = s1.tile([128, NCT, 6 * D], F32R, tag="wqkv", name="wqkv")
            nc.sync.dma_start(wqkv[:], wqkv_t)
            wout = s1.tile([128, HPC, C], F32R, tag="wout", name="wout")
            nc.sync.dma_start(wout[:], wout_t)
            cos = s1.tile([128, T], F32, tag="cos", name="cos")
            nc.sync.dma_start(cos[:], cos_d)
            sin = s1.tile([128, T], F32, tag="sin", name="sin")
            nc.sync.dma_start(sin[:], sin_d)
            mask = s1.tile([128, 128], F32, tag="mask", name="mask")
            nc.sync.dma_start(mask[:], mask_d)
            ones = s1.tile([128, 1], F32R, tag="ones", name="ones")
            nc.sync.dma_start(ones[:], ones_d)

            def proj_block(b, j, ou_sb):
                """Project tq block j of batch b through w_out and DMA out."""
                for tt in range(4):  # 4 tq tiles of 128 inside the block
                    for cb in range(NBLK):
                        yps = ps2.tile([128, 512], F32, tag="blk", name="yps")
                        for h in range(HPC):
                            nc.tensor.matmul(
                                yps[:],
                                ou_sb[h][:, ds(tt * 128, 128)],
                                wout[:, h, ds(cb * 512, 512)],
                                start=(h == 0),
                                stop=(h == HPC - 1),
                            )
                        ysb = sy.tile([128, 512], F32, tag="y", name="ysb")
                        nc.scalar.activation(ysb[:], yps[:], Copy)
                        nc.sync.dma_start(
                            y_d[
                                ds(b * T + j * 512 + tt * 128, 128),
                                ds(cb * 512, 512),
                            ],
                            ysb[:],
                        )

            for b in range(B):
                # ================= qkv projection + RoPE =================
                qk = [
                    s1.tile([128, T], F32R, tag=f"qk{i}", name=f"qk{i}")
                    for i in range(4)  # q0 q1 k0 k1
                ]
                v_sb = s1.tile([128, NTK, HPC * D], F32R, tag="v", name="v_sb")

                for blk in range(2 * NBLK):  # 8 token blocks of 256
                    tok0 = b * T + blk * 256
                    xt = s2.tile([128, NCT, 256], F32R, tag="xt", name="xt")
                    nc.sync.dma_start(xt[:], xt_t[:, :, ds(tok0, 256)])

                    for ht in range(4):  # q0 q1 k0 k1
                        ps = ps2.tile([128, 256], F32, tag="qk_ps", name="qk_ps")
                        for ct in range(NCT):
                            nc.tensor.matmul(
                                ps[:],
                                wqkv[:, ct, ds(ht * D, D)],
                                xt[:, ct, :],
                                start=(ct == 0),
                                stop=(ct == NCT - 1),
                            )
                        # RoPE: qk_blk = ps*cos + swap_halves(ps)*sin_signed
                        cs = cos[:, ds(blk * 256, 256)]
                        sn = sin[:, ds(blk * 256, 256)]
                        shuf = s2.tile([128, 256], F32, tag="shuf", name="shuf", bufs=2)
                        nc.scalar.copy(shuf[0:64, :], ps[64:128, :])
                        nc.scalar.copy(shuf[64:128, :], ps[0:64, :])
                        nc.vector.tensor_mul(shuf[:], shuf[:], sn)
                        dst = qk[ht][:, ds(blk * 256, 256)]
                        nc.vector.tensor_mul(dst, ps[:], cs)
                        nc.vector.tensor_add(dst, dst, shuf[:])

                    vps = ps1.tile([128, 2, HPC * D], F32, tag="v_ps", name="v_ps")
                    for sub in range(2):
                        for ct in range(NCT):
                            nc.tensor.matmul(
                                vps[:, sub, :],
                                xt[:, ct, ds(sub * 128, 128)],
                                wqkv[:, ct, ds(4 * D, HPC * D)],
                                start=(ct == 0),
                                stop=(ct == NCT - 1),
                            )
                    nc.scalar.copy(
                        v_sb[:, ds(blk * 2, 2), :].rearrange("p a b -> p (a b)"),
                        vps[:].rearrange("p a b -> p (a b)"),
                    )

                # ================= attention (+ inlined projection) ======
                prev_ou = None
                for j in range(NBLK):
                    ou_sb = []
                    for h in range(HPC):
                        qT, kT = qk[h], qk[2 + h]
                        ntk = 4 * j + 4
                        ou_ps = ps2.tile([128, 512], F32, tag="ou_ps", name="ou_ps")
                        cs_ps = ps1.tile([1, 512], F32, tag="cs_ps", name="cs_ps")

                        def scores(i):
                            sp = ps2.tile([128, 512], F32, tag="blk", name="sp")
                            nc.tensor.matmul(
                                sp[:],
                                kT[:, ds(i * 128, 128)],
                                qT[:, ds(j * 512, 512)],
                                start=True,
                                stop=True,
                            )
                            return sp

                        def exp_of(i, sp):
                            e = se.tile([128, 512], F32R, tag="e", name="e")
                            rr = i - 4 * j
                            if rr < 0:  # full block, all causal-valid
                                nc.scalar.activation(e[:], sp[:], Exp, scale=EXP_SCALE)
                            else:
                                nc.vector.tensor_add(
                                    sp[:, ds(rr * 128, 128)],
                                    sp[:, ds(rr * 128, 128)],
                                    mask[:],
                                )
                                if rr > 0:  # fully-masked columns -> 0
                                    nc.scalar.activation(
                                        e[:, ds(0, rr * 128)],
                                        sp[:, ds(0, rr * 128)],
                                        Copy,
                                        scale=0.0,
                                    )
                                nc.scalar.activation(
                                    e[:, ds(rr * 128, 512 - rr * 128)],
                                    sp[:, ds(rr * 128, 512 - rr * 128)],
                                    Exp,
                                    scale=EXP_SCALE,
                                )
                            return e

                        sp = scores(0)
                        for i in range(ntk):
                            e = exp_of(i, sp)
                            sp = scores(i + 1) if i + 1 < ntk else None
                            nc.tensor.matmul(
                                ou_ps[:],
                                v_sb[:, i, ds(h * D, D)],
                                e[:],
                                start=(i == 0),
                                stop=(i == ntk - 1),
                            )
                            nc.tensor.matmul(
                                cs_ps[:],
                                ones[:],
                                e[:],
                                start=(i == 0),
                                stop=(i == ntk - 1),
                            )

                        bc = s2.tile([128, 512], F32, tag="bc", name="bc", bufs=1)
                        nc.vector.reciprocal(bc[0:1, :], cs_ps[:])
                        nc.gpsimd.partition_broadcast(bc[:], bc[0:1, :])
                        ousb = sou.tile([128, 512], F32R, tag="ou", name="ousb")
                        nc.vector.tensor_mul(ousb[:], ou_ps[:], bc[:])
                        ou_sb.append(ousb)

                        if h == 0 and j > 0:
                            proj_block(b, j - 1, prev_ou)
                    prev_ou = ou_sb
                proj_block(b, NBLK - 1, prev_ou)

    nc.compile()
    return nc


def _host_prep(x, w_qkv, w_out, cos, sin):
    x = np.asarray(x, dtype=np.float32)
    w_qkv = np.asarray(w_qkv, dtype=np.float32)
    w_out = np.asarray(w_out, dtype=np.float32)
    cos = np.asarray(cos, dtype=np.float32)
    sin = np.asarray(sin, dtype=np.float32)

    xt = np.ascontiguousarray(x.reshape(S, C).T)  # [C, S]
    cos2t = np.ascontiguousarray(np.concatenate([cos, cos], axis=1).T)  # [D, T]
    sin2t = np.ascontiguousarray(np.concatenate([-sin, sin], axis=1).T)
    # maskadd[tk_local, tq_local]: NEG where tk > tq (strictly lower triangle)
    maskadd = np.tril(np.full((128, 128), NEG, dtype=np.float32), k=-1)
    ones = np.ones((128, 1), dtype=np.float32)

    in_maps = []
    for c in range(NCORES):
        h0 = c * HPC
        cols = []
        for qkv_i in range(3):
            for h in range(HPC):
                base = qkv_i * C + (h0 + h) * D
                cols.append(w_qkv[:, base : base + D])
        wqkv_c = np.ascontiguousarray(np.concatenate(cols, axis=1))  # [C, 768]
        wout_c = np.ascontiguousarray(w_out[h0 * D : (h0 + HPC) * D, :])  # [256, C]
        in_maps.append(
            {
                "xt": xt,
                "wqkv": wqkv_c,
                "wout": wout_c,
                "cos2t": cos2t,
                "sin2t": sin2t,
                "maskadd": maskadd,
                "ones_in": ones,
            }
        )
    return in_maps


def kernel(x, w_qkv, w_out, cos, sin):
    if "nc" not in _CACHE:
        _CACHE["nc"] = build_nc()
    nc = _CACHE["nc"]
    in_maps = _host_prep(x, w_qkv, w_out, cos, sin)
    res = run_bass_kernel_spmd(nc, in_maps, core_ids=list(range(NCORES)))
    acc = np.zeros((S, C), dtype=np.float64)
    for rmap in res.results:
        acc += rmap["y"].astype(np.float64)
    return acc.astype(np.float32).reshape(B, T, C)
